# revision 10
# baseline (speedup 1.0000x reference)
"""Trainium2 Bass kernel for nn_MoE_58772332479041 (8-expert top-2 SwiGLU MoE).

Strategy (expert-parallel, per the sharding hint):
  - Router (16384x2048 @ 2048x8 + softmax/top-2) runs on host: it is 0.06% of
    the FLOPs. Token->expert assignment, stable sort by expert and the scatter
    combine are index bookkeeping, also host-side.
  - The 99.9% of FLOPs - the three 2048x2048 GEMMs per routed token - run on
    the 8 NeuronCores, one expert per core. Each core gets its expert's
    gathered+pre-scaled tokens (padded to a fixed capacity C) already
    transposed to [D, C] so the contraction dim lies on SBUF partitions, plus
    its expert's w1/w3/w2. On-core: h = silu(x@w1)*(x@w3); out = h@w2, fully
    tiled/pipelined with Tile.
  - Combine: every token appears in exactly top_k=2 routed rows, so the
    scatter-add is two gathers + one add on host.

Precision: x@w1 and x@w3 run in float32r (full fp32 operands, ~1e-4 matmul
error, 1 cycle/row at free-dim>=256); h@w2 runs in bf16 with w2 resident in
SBUF. Configurable via MOE_KCFG env ("f32r", "bf16", "mixed").
"""

import os
import sys
import types
import numpy as np
import ml_dtypes

import concourse.bass as bass
import concourse.mybir as mybir
import concourse.tile as tile
from concourse import bacc
from concourse.bass_utils import run_bass_kernel_spmd

# ---- problem constants (hardcoded; kernel.py must be self-contained) ----
E, TOPK = 8, 2
B, S, D, HID = 4, 4096, 2048, 2048
N = B * S
P = 128
F32 = mybir.dt.float32
BF16 = mybir.dt.bfloat16
F32R = mybir.dt.float32r

_CFGS = {
    #             mm1     mm2     TB   C
    "f32r":    ("f32r", "f32r", 512, 4352),
    "mixed":   ("f32r", "bf16", 512, 4352),
    "bf16":    ("bf16", "bf16", 768, 4352),
    "fp16":    ("fp16", "fp16", 512, 4352),
    "mixed16": ("f32r", "fp16", 512, 4352),
    "fb":      ("fp16", "bf16", 768, 4352),
    "bf":      ("bf16", "fp16", 768, 4352),
}
KCFG = os.environ.get("MOE_KCFG", "fp16")

# Results of the last device launch (exec_time_ns etc.), for test harnesses.
last_results = None


def _install_profile_shim():
    """Optional: register the NTFF profile hook so trace=True works under axon."""
    if "antenv.axon_hooks" in sys.modules:
        return
    try:
        from trn_agent_boot.trn_boot import _ntff_profile_via_ctypes
        hook = _ntff_profile_via_ctypes("/opt/axon/libaxon_pjrt.so")
        mod = types.ModuleType("antenv.axon_hooks")
        mod.get_axon_ntff_profile_hook = lambda: hook
        mod.set_axon_ntff_profile_hook = lambda h: None
        sys.modules["antenv.axon_hooks"] = mod
    except Exception:
        pass


_DTS = {"f32r": F32R, "bf16": BF16, "fp16": mybir.dt.float16}
_NPDTS = {"f32r": np.float32, "bf16": ml_dtypes.bfloat16, "fp16": np.float16}


def _dt(name):
    return _DTS[name]


def _np_dt(name):
    return _NPDTS[name]


def _mm_ap(ap, name):
    return ap


def _chunks(total, size):
    out, s = [], 0
    while s < total:
        out.append((s, min(size, total - s)))
        s += size
    return out


def build_expert_kernel(C, TB, mm1, mm2):
    """One NeuronCore program: dense SwiGLU expert over C (padded) tokens.

    Inputs (per core): xT [D, C] pre-scaled gathered tokens, transposed;
    w1 [D, H]; w3 [D, H]; w2 [H, D]. Output: out [C, D] f32.
    """
    sdt1, sdt2 = _dt(mm1), _dt(mm2)
    nc = bacc.Bacc(None, target_bir_lowering=False, debug=False)

    xT = nc.dram_tensor("xT", [D, C], sdt1, kind="ExternalInput")
    w1 = nc.dram_tensor("w1", [D, HID], sdt1, kind="ExternalInput")
    w3 = nc.dram_tensor("w3", [D, HID], sdt1, kind="ExternalInput")
    w2 = nc.dram_tensor("w2", [HID, D], sdt2, kind="ExternalInput")
    out = nc.dram_tensor("out", [C, D], F32, kind="ExternalOutput")

    KT = D // P          # contraction tiles for GEMM1/3
    HT = HID // P        # contraction tiles for GEMM2 / output tiles of h
    blocks = _chunks(C, TB)  # token blocks (last may be partial)
    DO = D // 512        # output column tiles
    w2_resident = (mm2 != "f32r")

    xT3 = xT[:].rearrange("(kt p) c -> p kt c", p=P)     # [128, KT, C]
    w13 = w1[:].rearrange("(kt p) h -> p kt h", p=P)     # [128, KT, H]
    w33 = w3[:].rearrange("(kt p) h -> p kt h", p=P)
    w23 = w2[:].rearrange("(ht p) d -> p ht d", p=P)     # [128, HT, D]
    out3 = out[:].rearrange("(mt p) d -> p mt d", p=P)   # [128, C/128, D]

    silu = mybir.ActivationFunctionType.Silu

    with tile.TileContext(nc) as tc:
        with (
            tc.tile_pool(name="xb", bufs=2) as xb_pool,
            tc.tile_pool(name="wt", bufs=6) as wt_pool,
            tc.tile_pool(name="w2p", bufs=1 if w2_resident else 2) as w2_pool,
            tc.tile_pool(name="hb", bufs=1) as h_pool,
            tc.tile_pool(name="s1", bufs=2) as s1_pool,
            tc.tile_pool(name="ob", bufs=3) as o_pool,
            tc.tile_pool(name="ps", bufs=6, space="PSUM") as ps_pool,
        ):
            w2res = None
            if w2_resident:
                w2res = w2_pool.tile([P, HT, D], sdt2, tag="w2res")

            for bi, (b0, bl) in enumerate(blocks):
                xb = xb_pool.tile([P, KT, bl], sdt1, tag="xb")
                nc.sync.dma_start(xb[:], xT3[:, :, b0:b0 + bl])
                hbuf = h_pool.tile([P, HT, bl], sdt2, tag="hb")
                tts = _chunks(bl, 512)

                # ---- h = silu(x @ w1) * (x @ w3), h laid out [H, tok] ----
                for ht in range(HT):
                    w1t = wt_pool.tile([P, KT, P], sdt1, tag="w1t")
                    nc.sync.dma_start(w1t[:], w13[:, :, ht * P:(ht + 1) * P])
                    w3t = wt_pool.tile([P, KT, P], sdt1, tag="w3t")
                    nc.sync.dma_start(w3t[:], w33[:, :, ht * P:(ht + 1) * P])
                    if w2_resident and bi == 0 and ht < DO:
                        # stage the resident w2 in 512-col chunks behind the
                        # first weight tiles so it doesn't stall the first MMs
                        nc.sync.dma_start(
                            w2res[:, :, ht * 512:(ht + 1) * 512],
                            w23[:, :, ht * 512:(ht + 1) * 512])
                    for (t0, tl) in tts:
                        ps1 = ps_pool.tile([P, 512], F32, tag="ps")
                        ps3 = ps_pool.tile([P, 512], F32, tag="ps")
                        for k in range(KT):
                            nc.tensor.matmul(
                                ps1[:, :tl],
                                _mm_ap(w1t[:, k, :], mm1),
                                _mm_ap(xb[:, k, t0:t0 + tl], mm1),
                                start=(k == 0), stop=(k == KT - 1),
                            )
                        for k in range(KT):
                            nc.tensor.matmul(
                                ps3[:, :tl],
                                _mm_ap(w3t[:, k, :], mm1),
                                _mm_ap(xb[:, k, t0:t0 + tl], mm1),
                                start=(k == 0), stop=(k == KT - 1),
                            )
                        s1 = s1_pool.tile([P, 512], F32, tag="s1")
                        nc.scalar.activation(s1[:, :tl], ps1[:, :tl], silu)
                        nc.vector.tensor_mul(
                            hbuf[:, ht, t0:t0 + tl], s1[:, :tl], ps3[:, :tl])

                # ---- out = h.T @ w2, out laid out [tok, D] ----
                for do in range(DO):
                    if w2_resident:
                        w2c = w2res[:, :, do * 512:(do + 1) * 512]
                    else:
                        w2t = w2_pool.tile([P, HT, 512], sdt2, tag="w2c")
                        nc.sync.dma_start(
                            w2t[:], w23[:, :, do * 512:(do + 1) * 512])
                        w2c = w2t[:, :, :]
                    for t2 in range(bl // P):
                        ps2 = ps_pool.tile([P, 512], F32, tag="ps")
                        for ht in range(HT):
                            nc.tensor.matmul(
                                ps2[:],
                                _mm_ap(hbuf[:, ht, t2 * P:(t2 + 1) * P], mm2),
                                _mm_ap(w2c[:, ht, :], mm2),
                                start=(ht == 0), stop=(ht == HT - 1),
                            )
                        osb = o_pool.tile([P, 512], F32, tag="ob")
                        nc.vector.tensor_copy(osb[:], ps2[:])
                        mt = b0 // P + t2
                        nc.sync.dma_start(out3[:, mt, do * 512:(do + 1) * 512],
                                          osb[:])
    nc.compile()
    return nc


_kernel_cache = {}


def _get_kernel(C, TB, mm1, mm2):
    key = (C, TB, mm1, mm2)
    if key not in _kernel_cache:
        _kernel_cache[key] = build_expert_kernel(C, TB, mm1, mm2)
    return _kernel_cache[key]


def kernel(x, w_router, w1, w2, w3):
    global last_results
    mm1, mm2, TB, C0 = _CFGS[KCFG]
    np1, np2 = _np_dt(mm1), _np_dt(mm2)

    x = np.asarray(x, dtype=np.float32)
    w_router = np.asarray(w_router, dtype=np.float32)
    w1 = np.asarray(w1, dtype=np.float32)
    w2 = np.asarray(w2, dtype=np.float32)
    w3 = np.asarray(w3, dtype=np.float32)

    xt = x.reshape(N, D)

    # ---- router (host) ----
    logits = xt @ w_router                               # [N, E] f32
    mx = logits.max(-1, keepdims=True)
    p = np.exp((logits - mx).astype(np.float64))
    scores = p / p.sum(-1, keepdims=True)                # [N, E] f64
    top_idx = np.argsort(-scores, axis=-1, kind="stable")[:, :TOPK]  # [N, 2]
    top_scores = np.take_along_axis(scores, top_idx, -1)             # f64

    flat_experts = top_idx.reshape(-1)                   # [N*2]
    sort_idx = np.argsort(flat_experts, kind="stable")
    token_indices = sort_idx // TOPK
    sorted_scores = top_scores.reshape(-1)[sort_idx]
    counts = np.bincount(flat_experts, minlength=E).astype(np.int64)
    offs = np.zeros(E + 1, np.int64)
    np.cumsum(counts, out=offs[1:])

    # capacity: smallest multiple of 128 holding the fullest expert (the
    # compile is per-process anyway, so a tight fit costs nothing extra)
    C = int(-(-counts.max() // 128) * 128)
    C = min(max(C, 512), max(C0, C))

    # ---- per-core dispatch (host gather + pre-scale + transpose) ----
    nc = _get_kernel(C, TB, mm1, mm2)
    in_maps = []
    for e in range(E):
        idx = token_indices[offs[e]:offs[e + 1]]
        sc = sorted_scores[offs[e]:offs[e + 1]].astype(np.float32)
        g = xt[idx] * sc[:, None]                        # [c_e, D] f32
        gT = np.zeros((D, C), np1)
        gT[:, :g.shape[0]] = g.T.astype(np1, copy=False)
        in_maps.append({
            "xT": gT,
            "w1": np.ascontiguousarray(w1[e]).astype(np1, copy=False),
            "w3": np.ascontiguousarray(w3[e]).astype(np1, copy=False),
            "w2": np.ascontiguousarray(w2[e]).astype(np2, copy=False),
        })

    # ---- expert GEMMs on the 8 NeuronCores ----
    if os.environ.get("BASS_TRACE"):
        _install_profile_shim()
    last_results = run_bass_kernel_spmd(nc, in_maps, core_ids=list(range(E)))
    routed = np.stack([r["out"] for r in last_results.results])  # [E, C, D]
    flat = routed.reshape(E * C, D)

    # ---- combine (host): each token = sum of its 2 routed rows ----
    pos_sorted = np.empty(N * TOPK, np.int64)
    for e in range(E):
        pos_sorted[offs[e]:offs[e + 1]] = e * C + np.arange(counts[e])
    pos_flat = np.empty(N * TOPK, np.int64)
    pos_flat[sort_idx] = pos_sorted
    pos = pos_flat.reshape(N, TOPK)
    out = flat[pos[:, 0]] + flat[pos[:, 1]]
    out = out.reshape(B, S, D).astype(np.float32, copy=False)

    # ---- aux losses (host) ----
    sum_scores = np.bincount(flat_experts, weights=top_scores.reshape(-1),
                             minlength=E)
    avg_top = sum_scores / np.maximum(counts.astype(np.float64), 1.0)
    dist = counts / counts.sum()
    lb_loss = np.float32((dist * avg_top).sum() * E)

    lse = mx[:, 0].astype(np.float64) + np.log(p.sum(-1))
    rz_loss = np.float32(np.mean(lse ** 2))

    return out, lb_loss, rz_loss


# revision 12
# speedup vs baseline: 1.0349x; 1.0349x over previous
"""Trainium2 Bass kernel for nn_MoE_58772332479041 (8-expert top-2 SwiGLU MoE).

Strategy (expert-parallel, per the sharding hint):
  - Router (16384x2048 @ 2048x8 + softmax/top-2) runs on host: it is 0.06% of
    the FLOPs. Token->expert assignment, stable sort by expert and the scatter
    combine are index bookkeeping, also host-side.
  - The 99.9% of FLOPs - the three 2048x2048 GEMMs per routed token - run on
    the 8 NeuronCores, one expert per core. Each core gets its expert's
    gathered+pre-scaled tokens (padded to a fixed capacity C) already
    transposed to [D, C] so the contraction dim lies on SBUF partitions, plus
    its expert's w1/w3/w2. On-core: h = silu(x@w1)*(x@w3); out = h@w2, fully
    tiled/pipelined with Tile.
  - Combine: every token appears in exactly top_k=2 routed rows, so the
    scatter-add is two gathers + one add on host.

Precision: x@w1 and x@w3 run in float32r (full fp32 operands, ~1e-4 matmul
error, 1 cycle/row at free-dim>=256); h@w2 runs in bf16 with w2 resident in
SBUF. Configurable via MOE_KCFG env ("f32r", "bf16", "mixed").
"""

import os
import sys
import types
import numpy as np
import ml_dtypes

import concourse.bass as bass
import concourse.mybir as mybir
import concourse.tile as tile
from concourse import bacc
from concourse.bass_utils import run_bass_kernel_spmd

# ---- problem constants (hardcoded; kernel.py must be self-contained) ----
E, TOPK = 8, 2
B, S, D, HID = 4, 4096, 2048, 2048
N = B * S
P = 128
F32 = mybir.dt.float32
BF16 = mybir.dt.bfloat16
F32R = mybir.dt.float32r

_CFGS = {
    #             mm1     mm2     TB   C
    "f32r":    ("f32r", "f32r", 512, 4352),
    "mixed":   ("f32r", "bf16", 512, 4352),
    "bf16":    ("bf16", "bf16", 768, 4352),
    "fp16":    ("fp16", "fp16", 512, 4352),
    "mixed16": ("f32r", "fp16", 512, 4352),
    "fb":      ("fp16", "bf16", 768, 4352),
    "bf":      ("bf16", "fp16", 768, 4352),
}
KCFG = os.environ.get("MOE_KCFG", "fp16")

# Results of the last device launch (exec_time_ns etc.), for test harnesses.
last_results = None


def _install_profile_shim():
    """Optional: register the NTFF profile hook so trace=True works under axon."""
    if "antenv.axon_hooks" in sys.modules:
        return
    try:
        from trn_agent_boot.trn_boot import _ntff_profile_via_ctypes
        hook = _ntff_profile_via_ctypes("/opt/axon/libaxon_pjrt.so")
        mod = types.ModuleType("antenv.axon_hooks")
        mod.get_axon_ntff_profile_hook = lambda: hook
        mod.set_axon_ntff_profile_hook = lambda h: None
        sys.modules["antenv.axon_hooks"] = mod
    except Exception:
        pass


_DTS = {"f32r": F32R, "bf16": BF16, "fp16": mybir.dt.float16}
_NPDTS = {"f32r": np.float32, "bf16": ml_dtypes.bfloat16, "fp16": np.float16}


def _dt(name):
    return _DTS[name]


def _np_dt(name):
    return _NPDTS[name]


def _mm_ap(ap, name):
    return ap


def _chunks(total, size, merge_small=0):
    out, s = [], 0
    while s < total:
        out.append((s, min(size, total - s)))
        s += size
    if merge_small and len(out) > 1 and out[-1][1] < merge_small:
        (s0, l0), (s1, l1) = out[-2], out[-1]
        out[-2:] = [(s0, l0 + l1)]
    return out


def build_expert_kernel(C, TB, mm1, mm2):
    """One NeuronCore program: dense SwiGLU expert over C (padded) tokens.

    Inputs (per core): xT [D, C] pre-scaled gathered tokens, transposed;
    w1 [D, H]; w3 [D, H]; w2 [H, D]. Output: out [C, D] f32.
    """
    sdt1, sdt2 = _dt(mm1), _dt(mm2)
    nc = bacc.Bacc(None, target_bir_lowering=False, debug=False)

    xT = nc.dram_tensor("xT", [D, C], sdt1, kind="ExternalInput")
    w1 = nc.dram_tensor("w1", [D, HID], sdt1, kind="ExternalInput")
    w3 = nc.dram_tensor("w3", [D, HID], sdt1, kind="ExternalInput")
    w2 = nc.dram_tensor("w2", [HID, D], sdt2, kind="ExternalInput")
    out = nc.dram_tensor("out", [C, D], F32, kind="ExternalOutput")

    KT = D // P          # contraction tiles for GEMM1/3
    HT = HID // P        # contraction tiles for GEMM2 / output tiles of h
    blocks = _chunks(C, TB, merge_small=256)  # token blocks; tiny tail merged
    DO = D // 512        # output column tiles
    w2_resident = (mm2 != "f32r")

    xT3 = xT[:].rearrange("(kt p) c -> p kt c", p=P)     # [128, KT, C]
    w13 = w1[:].rearrange("(kt p) h -> p kt h", p=P)     # [128, KT, H]
    w33 = w3[:].rearrange("(kt p) h -> p kt h", p=P)
    w23 = w2[:].rearrange("(ht p) d -> p ht d", p=P)     # [128, HT, D]
    out3 = out[:].rearrange("(mt p) d -> p mt d", p=P)   # [128, C/128, D]

    silu = mybir.ActivationFunctionType.Silu

    with tile.TileContext(nc) as tc:
        with (
            tc.tile_pool(name="xb", bufs=2) as xb_pool,
            tc.tile_pool(name="wt", bufs=6) as wt_pool,
            tc.tile_pool(name="w2p", bufs=1 if w2_resident else 2) as w2_pool,
            tc.tile_pool(name="hb", bufs=1) as h_pool,
            tc.tile_pool(name="s1", bufs=2) as s1_pool,
            tc.tile_pool(name="ob", bufs=3) as o_pool,
            tc.tile_pool(name="ps", bufs=6, space="PSUM") as ps_pool,
        ):
            w2res = None
            if w2_resident:
                w2res = w2_pool.tile([P, HT, D], sdt2, tag="w2res")

            for bi, (b0, bl) in enumerate(blocks):
                xb = xb_pool.tile([P, KT, bl], sdt1, tag="xb")
                nc.sync.dma_start(xb[:], xT3[:, :, b0:b0 + bl])
                hbuf = h_pool.tile([P, HT, bl], sdt2, tag="hb")
                tts = _chunks(bl, 512)  # matmul free-dim chunks

                # ---- h = silu(x @ w1) * (x @ w3), h laid out [H, tok] ----
                for ht in range(HT):
                    w1t = wt_pool.tile([P, KT, P], sdt1, tag="w1t")
                    nc.sync.dma_start(w1t[:], w13[:, :, ht * P:(ht + 1) * P])
                    w3t = wt_pool.tile([P, KT, P], sdt1, tag="w3t")
                    nc.sync.dma_start(w3t[:], w33[:, :, ht * P:(ht + 1) * P])
                    if w2_resident and bi == 0 and ht < DO:
                        # stage the resident w2 in 512-col chunks behind the
                        # first weight tiles so it doesn't stall the first MMs
                        nc.sync.dma_start(
                            w2res[:, :, ht * 512:(ht + 1) * 512],
                            w23[:, :, ht * 512:(ht + 1) * 512])
                    for (t0, tl) in tts:
                        ps1 = ps_pool.tile([P, 512], F32, tag="ps")
                        ps3 = ps_pool.tile([P, 512], F32, tag="ps")
                        for k in range(KT):
                            nc.tensor.matmul(
                                ps1[:, :tl],
                                _mm_ap(w1t[:, k, :], mm1),
                                _mm_ap(xb[:, k, t0:t0 + tl], mm1),
                                start=(k == 0), stop=(k == KT - 1),
                            )
                        for k in range(KT):
                            nc.tensor.matmul(
                                ps3[:, :tl],
                                _mm_ap(w3t[:, k, :], mm1),
                                _mm_ap(xb[:, k, t0:t0 + tl], mm1),
                                start=(k == 0), stop=(k == KT - 1),
                            )
                        s1 = s1_pool.tile([P, 512], F32, tag="s1")
                        nc.scalar.activation(s1[:, :tl], ps1[:, :tl], silu)
                        nc.vector.tensor_mul(
                            hbuf[:, ht, t0:t0 + tl], s1[:, :tl], ps3[:, :tl])

                # ---- out = h.T @ w2, out laid out [tok, D] ----
                for do in range(DO):
                    if w2_resident:
                        w2c = w2res[:, :, do * 512:(do + 1) * 512]
                    else:
                        w2t = w2_pool.tile([P, HT, 512], sdt2, tag="w2c")
                        nc.sync.dma_start(
                            w2t[:], w23[:, :, do * 512:(do + 1) * 512])
                        w2c = w2t[:, :, :]
                    for t2 in range(bl // P):
                        ps2 = ps_pool.tile([P, 512], F32, tag="ps")
                        for ht in range(HT):
                            nc.tensor.matmul(
                                ps2[:],
                                _mm_ap(hbuf[:, ht, t2 * P:(t2 + 1) * P], mm2),
                                _mm_ap(w2c[:, ht, :], mm2),
                                start=(ht == 0), stop=(ht == HT - 1),
                            )
                        osb = o_pool.tile([P, 512], F32, tag="ob")
                        nc.vector.tensor_copy(osb[:], ps2[:])
                        mt = b0 // P + t2
                        nc.sync.dma_start(out3[:, mt, do * 512:(do + 1) * 512],
                                          osb[:])
    nc.compile()
    return nc


_kernel_cache = {}


def _get_kernel(C, TB, mm1, mm2):
    key = (C, TB, mm1, mm2)
    if key not in _kernel_cache:
        _kernel_cache[key] = build_expert_kernel(C, TB, mm1, mm2)
    return _kernel_cache[key]


def kernel(x, w_router, w1, w2, w3):
    global last_results
    mm1, mm2, TB, C0 = _CFGS[KCFG]
    np1, np2 = _np_dt(mm1), _np_dt(mm2)

    x = np.asarray(x, dtype=np.float32)
    w_router = np.asarray(w_router, dtype=np.float32)
    w1 = np.asarray(w1, dtype=np.float32)
    w2 = np.asarray(w2, dtype=np.float32)
    w3 = np.asarray(w3, dtype=np.float32)

    xt = x.reshape(N, D)

    # ---- router (host) ----
    logits = xt @ w_router                               # [N, E] f32
    mx = logits.max(-1, keepdims=True)
    p = np.exp((logits - mx).astype(np.float64))
    scores = p / p.sum(-1, keepdims=True)                # [N, E] f64
    top_idx = np.argsort(-scores, axis=-1, kind="stable")[:, :TOPK]  # [N, 2]
    top_scores = np.take_along_axis(scores, top_idx, -1)             # f64

    flat_experts = top_idx.reshape(-1)                   # [N*2]
    sort_idx = np.argsort(flat_experts, kind="stable")
    token_indices = sort_idx // TOPK
    sorted_scores = top_scores.reshape(-1)[sort_idx]
    counts = np.bincount(flat_experts, minlength=E).astype(np.int64)
    offs = np.zeros(E + 1, np.int64)
    np.cumsum(counts, out=offs[1:])

    # capacity: smallest multiple of 128 holding the fullest expert (the
    # compile is per-process anyway, so a tight fit costs nothing extra)
    C = int(-(-counts.max() // 128) * 128)
    C = min(max(C, 512), max(C0, C))

    # ---- per-core dispatch (host gather + pre-scale + transpose) ----
    nc = _get_kernel(C, TB, mm1, mm2)
    in_maps = []
    for e in range(E):
        idx = token_indices[offs[e]:offs[e + 1]]
        sc = sorted_scores[offs[e]:offs[e + 1]].astype(np.float32)
        g = xt[idx] * sc[:, None]                        # [c_e, D] f32
        gT = np.zeros((D, C), np1)
        gT[:, :g.shape[0]] = g.T.astype(np1, copy=False)
        in_maps.append({
            "xT": gT,
            "w1": np.ascontiguousarray(w1[e]).astype(np1, copy=False),
            "w3": np.ascontiguousarray(w3[e]).astype(np1, copy=False),
            "w2": np.ascontiguousarray(w2[e]).astype(np2, copy=False),
        })

    # ---- expert GEMMs on the 8 NeuronCores ----
    if os.environ.get("BASS_TRACE"):
        _install_profile_shim()
    last_results = run_bass_kernel_spmd(nc, in_maps, core_ids=list(range(E)))
    routed = np.stack([r["out"] for r in last_results.results])  # [E, C, D]
    flat = routed.reshape(E * C, D)

    # ---- combine (host): each token = sum of its 2 routed rows ----
    pos_sorted = np.empty(N * TOPK, np.int64)
    for e in range(E):
        pos_sorted[offs[e]:offs[e + 1]] = e * C + np.arange(counts[e])
    pos_flat = np.empty(N * TOPK, np.int64)
    pos_flat[sort_idx] = pos_sorted
    pos = pos_flat.reshape(N, TOPK)
    out = flat[pos[:, 0]] + flat[pos[:, 1]]
    out = out.reshape(B, S, D).astype(np.float32, copy=False)

    # ---- aux losses (host) ----
    sum_scores = np.bincount(flat_experts, weights=top_scores.reshape(-1),
                             minlength=E)
    avg_top = sum_scores / np.maximum(counts.astype(np.float64), 1.0)
    dist = counts / counts.sum()
    lb_loss = np.float32((dist * avg_top).sum() * E)

    lse = mx[:, 0].astype(np.float64) + np.log(p.sum(-1))
    rz_loss = np.float32(np.mean(lse ** 2))

    return out, lb_loss, rz_loss
